# revision 1
# baseline (speedup 1.0000x reference)
"""LSTM decoder (constant input per step, ragged lengths) on 8 TRN2 cores.

Math (per batch element b, for t < seq_len[b]):
    x_proj = Z @ W_ih.T + b_ih + b_hh            (constant over time)
    gates_t = x_proj + h_t @ W_hh.T
    i,f,g,o = split(gates_t); c = sig(f)*c + sig(i)*tanh(g); h = sig(o)*tanh(c)
    ys[b, t] = h_{t+1}

The recurrence is chaotic: bf16/tf32-class rounding of h or W diverges from the
fp32 reference by O(1) after ~500 steps, so products must be fp32-exact.

Device strategy (data-parallel over batch, 16 sequences per core):
  * Streaming matmul form: stationary = h.T chunks [128, 16], moving = W_hh.T
    column blocks [128, 512].  Native fp32 matmul costs 4 cycles/row; instead
    both operands are Veltkamp-split into two ~12-bit-mantissa pieces which
    float32r (1 cycle/row at N>=512) multiplies EXACTLY, and the product is
    reconstructed in 3 accumulating passes (hi*hi + hi*lo + lo*hi, fp32 PSUM):
    verified 1.3e-7 relative vs fp64 on hardware, 25% less PE time than fp32.
  * Gates come out in "layout 1": [batch(16) partitions, gate free].  W columns
    are reordered host-side so N-chunk n holds the i|f|g|o gates of hidden
    block n (128 units): each chunk's elementwise finishes early, its h block
    is PE-transposed and split for the next step's stationary.
  * x_proj computed once on device the same way; bias added from a host tile.
"""

import numpy as np

import concourse.bass as bass
import concourse.tile as tile
from concourse import bacc, mybir
from concourse.bass_utils import run_bass_kernel_spmd

B, F, H, TMAX = 128, 128, 512, 512
N_CORES = 8
BL = B // N_CORES          # local batch = 16
NB = 4                     # hidden blocks of 128 (= N chunks and K chunks)
T_STEPS = TMAX - 1         # seq_len < 512, so at most 511 steps matter
SPLIT_C = float(2.0 ** 12 + 1)

FP32 = mybir.dt.float32
FP32R = mybir.dt.float32r
AF = mybir.ActivationFunctionType


def _split12(x):
    x = x.astype(np.float32)
    v = (x * np.float32(SPLIT_C)).astype(np.float32)
    hi = (v - (v - x).astype(np.float32)).astype(np.float32)
    lo = (x - hi).astype(np.float32)
    return hi, lo


def build_lstm_nc(t_steps: int = T_STEPS):
    """Build + compile the per-core Bass program (SPMD: same NEFF, 8 cores)."""
    nc = bacc.Bacc("TRN2", target_bir_lowering=False, debug=False)

    # W_hh.T, columns reordered and hi/lo split:
    #   wr*[:, k*2048 + n*512 + g*128 + q] = split(W_hh[g*512+128n+q, 128k+p])
    wrh_d = nc.dram_tensor("wrh", [128, NB * 2048], FP32R, kind="ExternalInput")
    wrl_d = nc.dram_tensor("wrl", [128, NB * 2048], FP32R, kind="ExternalInput")
    # W_ih.T with the same column reorder, hi/lo (single K chunk, F=128)
    wih_d = nc.dram_tensor("wih", [128, 2048], FP32R, kind="ExternalInput")
    wil_d = nc.dram_tensor("wil", [128, 2048], FP32R, kind="ExternalInput")
    z_d = nc.dram_tensor("z", [128, 2 * BL], FP32R, kind="ExternalInput")  # [hi|lo]
    bias_d = nc.dram_tensor("bias", [BL, 2048], FP32, kind="ExternalInput")
    eye_d = nc.dram_tensor("eye", [128, 128], FP32, kind="ExternalInput")
    ys_d = nc.dram_tensor("ys", [t_steps, BL, H], FP32, kind="ExternalOutput")

    with tile.TileContext(nc) as tc:
        with (
            tc.tile_pool(name="const", bufs=1) as constp,
            tc.tile_pool(name="state", bufs=1) as statep,
            tc.tile_pool(name="work", bufs=3) as workp,
            tc.tile_pool(name="hout", bufs=4) as houtp,
            tc.tile_pool(name="ps", bufs=3, space="PSUM") as psp,
            tc.tile_pool(name="pst", bufs=2, space="PSUM") as pstp,
        ):
            # --- constants ---
            wrh = constp.tile([128, NB * 2048], FP32R)
            nc.sync.dma_start(wrh[:], wrh_d.ap())
            wrl = constp.tile([128, NB * 2048], FP32R)
            nc.sync.dma_start(wrl[:], wrl_d.ap())
            wih = constp.tile([128, 2048], FP32R)
            nc.sync.dma_start(wih[:], wih_d.ap())
            wil = constp.tile([128, 2048], FP32R)
            nc.sync.dma_start(wil[:], wil_d.ap())
            z2 = constp.tile([128, 2 * BL], FP32R)
            nc.sync.dma_start(z2[:], z_d.ap())
            bias = constp.tile([BL, 2048], FP32)
            nc.sync.dma_start(bias[:BL, :], bias_d.ap())
            eye = constp.tile([128, 128], FP32)
            nc.sync.dma_start(eye[:], eye_d.ap())

            # --- x_proj (once): 3-pass exact product + bias ---
            xp1 = constp.tile([BL, 2048], FP32)
            z_hi = z2[:, :BL]
            z_lo = z2[:, BL:]
            for n in range(NB):
                xps = psp.tile([BL, 512], FP32, tag="xps")
                wi_h = wih[:, n * 512 : (n + 1) * 512]
                wi_l = wil[:, n * 512 : (n + 1) * 512]
                nc.tensor.matmul(xps[:BL, :], z_hi, wi_h, start=True, stop=False)
                nc.tensor.matmul(xps[:BL, :], z_hi, wi_l, start=False, stop=False)
                nc.tensor.matmul(xps[:BL, :], z_lo, wi_h, start=False, stop=True)
                nc.vector.tensor_add(
                    xp1[:BL, n * 512 : (n + 1) * 512],
                    xps[:BL, :],
                    bias[:BL, n * 512 : (n + 1) * 512],
                )

            # --- state ---
            c1 = statep.tile([BL, H], FP32)          # cell, layout 1
            nc.vector.memset(c1[:BL, :], 0.0)
            # h.T hi/lo state, packed: chunk k valid at cols [32k, 32k+16)
            hTh = [
                statep.tile([128, 128], FP32R, tag=f"hTh{j}", name=f"hTh{j}")
                for j in range(2)
            ]
            hTl = [
                statep.tile([128, 128], FP32R, tag=f"hTl{j}", name=f"hTl{j}")
                for j in range(2)
            ]
            zf = statep.tile([128, 128], FP32)
            nc.vector.memset(zf[:], 0.0)
            nc.vector.tensor_copy(hTh[0][:], zf[:])
            nc.vector.tensor_copy(hTl[0][:], zf[:])

            # --- recurrence ---
            # Block order (3,0,1,2) both for processing and k-accumulation: the
            # last-computed block's h chunk is the last one the next step's
            # first psum group consumes, hiding the end-of-step tail.
            ORDER = (3, 0, 1, 2)
            for t in range(t_steps):
                hh_p, hl_p = hTh[t % 2], hTl[t % 2]
                hh_n, hl_n = hTh[(t + 1) % 2], hTl[(t + 1) % 2]
                # h packed: block n at partitions [32n, 32n+16)
                h1 = houtp.tile([128, 128], FP32, tag="h1")
                nc.vector.memset(h1[:], 0.0)
                for ni, n in enumerate(ORDER):
                    ps = psp.tile([BL, 512], FP32, tag="gates")
                    for ki, k in enumerate(ORDER):
                        w_h = wrh[:, k * 2048 + n * 512 : k * 2048 + (n + 1) * 512]
                        w_l = wrl[:, k * 2048 + n * 512 : k * 2048 + (n + 1) * 512]
                        s_h = hh_p[:, k * 32 : k * 32 + BL]
                        s_l = hl_p[:, k * 32 : k * 32 + BL]
                        nc.tensor.matmul(ps[:BL, :], s_h, w_h,
                                         start=(ki == 0), stop=False)
                        nc.tensor.matmul(ps[:BL, :], s_h, w_l, start=False, stop=False)
                        nc.tensor.matmul(ps[:BL, :], s_l, w_h,
                                         start=False, stop=(ki == NB - 1))
                    # elementwise for hidden block n: chunk = [i|f|g|o] x 128
                    ga = workp.tile([BL, 512], FP32, tag="ga")
                    nc.vector.tensor_add(
                        ga[:BL, :], ps[:BL, :], xp1[:BL, n * 512 : (n + 1) * 512]
                    )
                    act = workp.tile([BL, 512], FP32, tag="act")
                    nc.scalar.activation(act[:BL, 0:256], ga[:BL, 0:256], AF.Sigmoid)
                    nc.scalar.activation(act[:BL, 256:384], ga[:BL, 256:384], AF.Tanh)
                    nc.scalar.activation(act[:BL, 384:512], ga[:BL, 384:512], AF.Sigmoid)
                    i_s = act[:BL, 0:128]
                    f_s = act[:BL, 128:256]
                    g_s = act[:BL, 256:384]
                    o_s = act[:BL, 384:512]
                    cn = c1[:BL, n * 128 : (n + 1) * 128]
                    t1 = workp.tile([BL, 128], FP32, tag="t1")
                    nc.vector.tensor_mul(t1[:BL, :], i_s, g_s)
                    nc.vector.tensor_mul(cn, f_s, cn)
                    nc.vector.tensor_add(cn, cn, t1[:BL, :])
                    tct = workp.tile([BL, 128], FP32, tag="tct")
                    nc.scalar.activation(tct[:BL, :], cn, AF.Tanh)
                    hn = h1[32 * n : 32 * n + BL, :]
                    nc.vector.tensor_mul(hn, o_s, tct[:BL, :])
                    nc.sync.dma_start(
                        ys_d.ap()[t, :, n * 128 : (n + 1) * 128], hn
                    )
                # one packed transpose for all 4 blocks, then hi/lo split
                psT = pstp.tile([128, 128], FP32, tag="psT")
                nc.tensor.transpose(psT[:, :], h1[:, :], eye[:])
                nc.vector.tensor_copy(hh_n[:], psT[:, :])
                nc.vector.tensor_sub(hl_n[:], psT[:, :], hh_n[:])

    nc.compile()
    return nc


def _prep_host_inputs(Z, seq_len, W_ih, W_hh, b_ih, b_hh):
    """Per-core in_maps with device-native layouts."""
    WT = np.ascontiguousarray(W_hh.astype(np.float32).T)      # [H, 4H] (hid_in, gate)
    WIT = np.ascontiguousarray(W_ih.astype(np.float32).T)     # [F, 4H]
    bias = (b_ih.astype(np.float32) + b_hh.astype(np.float32))

    # column reorder: col = n*512 + g*128 + q  <->  gate index g*512 + 128n + q
    n_i = np.arange(2048)
    nn, rem = np.divmod(n_i, 512)
    gg, qq = np.divmod(rem, 128)
    colmap = gg * H + 128 * nn + qq                           # [2048]

    wr_np = np.empty((128, NB * 2048), dtype=np.float32)
    for k in range(NB):
        wr_np[:, k * 2048 : (k + 1) * 2048] = WT[k * 128 : (k + 1) * 128, colmap]
    wrh_np, wrl_np = _split12(wr_np)
    wih_np, wil_np = _split12(np.ascontiguousarray(WIT[:, colmap]))
    bias_np = np.broadcast_to(bias[colmap], (BL, 2048)).copy()
    eye_np = np.eye(128, dtype=np.float32)

    in_maps = []
    for c in range(N_CORES):
        zc = np.ascontiguousarray(Z[c * BL : (c + 1) * BL].astype(np.float32).T)
        z_hi, z_lo = _split12(zc)
        z_np = np.concatenate([z_hi, z_lo], axis=1)
        in_maps.append(
            {"wrh": wrh_np, "wrl": wrl_np, "wih": wih_np, "wil": wil_np,
             "z": z_np, "bias": bias_np, "eye": eye_np}
        )
    return in_maps


_NC_CACHE = {}


def get_nc(t_steps: int = T_STEPS):
    if t_steps not in _NC_CACHE:
        _NC_CACHE[t_steps] = build_lstm_nc(t_steps)
    return _NC_CACHE[t_steps]


def kernel(Z, seq_len, W_ih, W_hh, b_ih, b_hh, _trace=False, _tmpdir=None):
    nc = get_nc()
    in_maps = _prep_host_inputs(Z, seq_len, W_ih, W_hh, b_ih, b_hh)
    res = run_bass_kernel_spmd(
        nc, in_maps, core_ids=list(range(N_CORES)), trace=_trace, tmpdir=_tmpdir
    )
    kernel.last_result = res

    out = np.zeros((B, TMAX, H), dtype=np.float32)
    for c in range(N_CORES):
        ys = res.results[c]["ys"]  # [T_STEPS, BL, H] — batch-major, natural hid order
        out[c * BL : (c + 1) * BL, :T_STEPS] = ys.transpose(1, 0, 2)
    mask = np.arange(TMAX, dtype=np.int64)[None, :] < seq_len.astype(np.int64)[:, None]
    out *= mask[:, :, None].astype(np.float32)
    return out



# revision 24
# speedup vs baseline: 1.2604x; 1.2604x over previous
"""LSTM decoder (constant input per step, ragged lengths) on 8 TRN2 cores.

Math (per batch element b, for t < seq_len[b]):
    x_proj = Z @ W_ih.T + b_ih + b_hh            (constant over time)
    gates_t = x_proj + h_t @ W_hh.T
    i,f,g,o = split(gates_t); c = sig(f)*c + sig(i)*tanh(g); h = sig(o)*tanh(c)
    ys[b, t] = h_{t+1}

The recurrence is chaotic: bf16/tf32-class rounding of h or W diverges from the
fp32 reference by O(1) after ~500 steps, so products must be fp32-exact.

Device strategy (data-parallel over batch, 16 sequences per core):
  * Streaming matmul: moving = W_hh.T column blocks [128, 512] (fp32r,
    1 cycle/row at N>=256), stationary = h.T chunks, Veltkamp-split into
    ~12-bit hi/lo pieces whose products are exact in fp32.
  * M-stacking: stationary packs [hi | zeros | lo] (48 of 128 PE columns),
    so one moving pass of wh yields hi*wh (psum rows 0:16) AND lo*wh (rows
    32:48); the wl pass adds hi*wl / lo*wl.  Full exact product in 2 moving
    passes instead of 3 -> 32 matmuls/step instead of 48.
  * Row halves merge: DMA evacuates the lo half (engines stay free) while
    DVE adds x_proj to the hi half, then one DVE add combines them.
  * Gate order i|f|o|g per 128-hidden block: one sigmoid over 384 cols +
    one tanh over 128.
  * Per-chunk PE transpose + hi/lo split so the next step's first matmul
    groups (k-ordered by availability) start while late chunks' elementwise
    still runs; emission is software-pipelined to avoid engine-FIFO
    head-of-line blocking.
"""

import numpy as np

import concourse.bass as bass
import concourse.tile as tile
from concourse import bacc, mybir
from concourse.bass_utils import run_bass_kernel_spmd

B, F, H, TMAX = 128, 128, 512, 512
N_CORES = 8
BL = B // N_CORES          # local batch = 16
NB = 4                     # hidden blocks of 128 (= N chunks and K chunks)
T_STEPS = TMAX - 1         # seq_len < 512, so at most 511 steps matter
SPLIT_C = float(2.0 ** 12 + 1)

FP32 = mybir.dt.float32
FP32R = mybir.dt.float32r
AF = mybir.ActivationFunctionType


def _split12(x):
    x = x.astype(np.float32)
    v = (x * np.float32(SPLIT_C)).astype(np.float32)
    hi = (v - (v - x).astype(np.float32)).astype(np.float32)
    lo = (x - hi).astype(np.float32)
    return hi, lo


def build_lstm_nc(t_steps: int = T_STEPS):
    """Build + compile the per-core Bass program (SPMD: same NEFF, 8 cores)."""
    nc = bacc.Bacc("TRN2", target_bir_lowering=False, debug=False)

    wrh_d = nc.dram_tensor("wrh", [128, NB * 2048], FP32R, kind="ExternalInput")
    wrl_d = nc.dram_tensor("wrl", [128, NB * 2048], FP32R, kind="ExternalInput")
    wih_d = nc.dram_tensor("wih", [128, 2048], FP32R, kind="ExternalInput")
    wil_d = nc.dram_tensor("wil", [128, 2048], FP32R, kind="ExternalInput")
    z_d = nc.dram_tensor("z", [128, 2 * BL], FP32R, kind="ExternalInput")  # [hi|lo]
    bias_d = nc.dram_tensor("bias", [BL, 2048], FP32, kind="ExternalInput")
    eye_d = nc.dram_tensor("eye", [128, 128], FP32, kind="ExternalInput")
    # ys stored hid-block-major: [t, k, b, q] with hid = 128k + q
    ys_d = nc.dram_tensor("ys", [t_steps, NB, BL, 128], FP32, kind="ExternalOutput")

    with tile.TileContext(nc) as tc:
        with (
            tc.tile_pool(name="const", bufs=1) as constp,
            tc.tile_pool(name="state", bufs=1) as statep,
            tc.tile_pool(name="work", bufs=4) as workp,
            tc.tile_pool(name="hout", bufs=4) as houtp,
            tc.tile_pool(name="ps", bufs=4, space="PSUM") as psp,
            tc.tile_pool(name="pst", bufs=2, space="PSUM") as pstp,
        ):
            # --- constants ---
            wrh = constp.tile([128, NB * 2048], FP32R)
            nc.sync.dma_start(wrh[:], wrh_d.ap())
            wrl = constp.tile([128, NB * 2048], FP32R)
            nc.sync.dma_start(wrl[:], wrl_d.ap())
            wih = constp.tile([128, 2048], FP32R)
            nc.sync.dma_start(wih[:], wih_d.ap())
            wil = constp.tile([128, 2048], FP32R)
            nc.sync.dma_start(wil[:], wil_d.ap())
            z2 = constp.tile([128, 2 * BL], FP32R)
            nc.sync.dma_start(z2[:], z_d.ap())
            bias = constp.tile([BL, 2048], FP32)
            nc.sync.dma_start(bias[:BL, :], bias_d.ap())
            eye = constp.tile([128, 128], FP32)
            nc.sync.dma_start(eye[:], eye_d.ap())

            # --- x_proj (once): 3-pass exact product + bias ---
            xp1 = constp.tile([BL, 2048], FP32)
            z_hi = z2[:, :BL]
            z_lo = z2[:, BL:]
            for n in range(NB):
                xps = psp.tile([48, 512], FP32, tag="gates", name="psg")
                wi_h = wih[:, n * 512 : (n + 1) * 512]
                wi_l = wil[:, n * 512 : (n + 1) * 512]
                nc.tensor.matmul(xps[:BL, :512], z_hi, wi_h, start=True, stop=False)
                nc.tensor.matmul(xps[:BL, :512], z_hi, wi_l, start=False, stop=False)
                nc.tensor.matmul(xps[:BL, :512], z_lo, wi_h, start=False, stop=True)
                nc.vector.tensor_add(
                    xp1[:BL, n * 512 : (n + 1) * 512],
                    xps[:BL, :512],
                    bias[:BL, n * 512 : (n + 1) * 512],
                )

            # --- state ---
            c1 = statep.tile([BL, H], FP32)          # cell, layout 1
            nc.vector.memset(c1[:BL, :], 0.0)
            # stationary state, double buffered: sP[p] [128, 192] fp32r,
            # chunk k at cols [48k, 48k+48): hi at +0:16, ZERO at +16:32
            # (so lo's psum rows land 32-aligned), lo at +32:48.
            sP = [
                statep.tile([128, 192], FP32R, tag=f"sP{j}", name=f"sP{j}")
                for j in range(2)
            ]
            zf = statep.tile([128, 192], FP32)
            nc.vector.memset(zf[:], 0.0)
            nc.vector.tensor_copy(sP[0][:], zf[:])
            nc.vector.tensor_copy(sP[1][:], zf[:])

            # --- recurrence (software-pipelined emission) ---
            ORDER = (3, 0, 1, 2)
            SLOT = {n: i for i, n in enumerate(ORDER)}  # h1 partition slot

            def emit_mm_pairs(ps, n, s_p, ks, start, stop):
                for ki, k in enumerate(ks):
                    w_h = wrh[:, k * 2048 + n * 512 : k * 2048 + (n + 1) * 512]
                    w_l = wrl[:, k * 2048 + n * 512 : k * 2048 + (n + 1) * 512]
                    s_k = s_p[:, k * 48 : k * 48 + 48]
                    nc.tensor.matmul(ps[:48, :], s_k, w_h,
                                     start=(start and ki == 0), stop=False)
                    nc.tensor.matmul(ps[:48, :], s_k, w_l, start=False,
                                     stop=(stop and ki == len(ks) - 1))

            def emit_evac(n, ps, fast):
                """PSUM -> activated gates in SBUF.  Act evacuates the lo half;
                the hi half is read once by DVE.  For non-critical chunks the
                lo+xp merge runs on gpsimd (SBUF-only) to spare DVE."""
                lo_sb = workp.tile([BL, 512], FP32, tag="lo", name="lo_sb")
                nc.scalar.activation(lo_sb[:BL, :], ps[32:48, :], AF.Copy)
                ga = workp.tile([BL, 512], FP32, tag="ga", name="ga")
                xpc = xp1[:BL, n * 512 : (n + 1) * 512]
                if fast:
                    nc.vector.tensor_add(ga[:BL, :], ps[0:BL, :], xpc)
                    nc.vector.tensor_add(ga[:BL, :], ga[:BL, :], lo_sb[:BL, :])
                else:
                    nc.gpsimd.tensor_add(lo_sb[:BL, :], lo_sb[:BL, :], xpc)
                    nc.vector.tensor_add(ga[:BL, :], ps[0:BL, :], lo_sb[:BL, :])
                act = workp.tile([BL, 512], FP32, tag="act", name="act")
                nc.scalar.activation(act[:BL, 0:384], ga[:BL, 0:384], AF.Sigmoid)
                nc.scalar.activation(act[:BL, 384:512], ga[:BL, 384:512], AF.Tanh)
                return act

            def emit_cchain(n, act, h1, on_dve):
                i_s = act[:BL, 0:128]
                f_s = act[:BL, 128:256]
                o_s = act[:BL, 256:384]
                g_s = act[:BL, 384:512]
                cn = c1[:BL, n * 128 : (n + 1) * 128]
                eng = nc.vector if on_dve else nc.gpsimd
                t1 = workp.tile([BL, 128], FP32, tag="t1", name="t1")
                eng.tensor_mul(t1[:BL, :], i_s, g_s)
                eng.tensor_mul(cn, f_s, cn)
                eng.tensor_add(cn, cn, t1[:BL, :])
                tct = workp.tile([BL, 128], FP32, tag="tct", name="tct")
                nc.scalar.activation(tct[:BL, :], cn, AF.Tanh)
                hn = h1[32 * SLOT[n] : 32 * SLOT[n] + BL, :]
                nc.vector.tensor_mul(hn, o_s, tct[:BL, :])

            def emit_t_single(n, h1, s_n):
                """Transpose one chunk (slot base 0 or 32) + hi/lo split."""
                base = 32 * SLOT[n]
                psT = pstp.tile([128, 64], FP32, tag="psT", name="psT")
                nc.tensor.transpose(
                    psT[:, 0:32], h1[base : base + 32, :],
                    eye[base : base + 32, base : base + 32],
                )
                hi = s_n[:, 48 * n : 48 * n + 16]
                lo = s_n[:, 48 * n + 32 : 48 * n + 48]
                nc.vector.tensor_copy(hi, psT[:, 0:BL])
                nc.vector.tensor_sub(lo, psT[:, 0:BL], hi)

            def emit_t_pair12(h1, s_n):
                """Transpose slots 2,3 (chunks ORDER[2]=1, ORDER[3]=2) at
                base 64 together, then split both with 2-level free APs."""
                n_a, n_b = ORDER[2], ORDER[3]
                assert n_a == 1 and n_b == 2
                psT = pstp.tile([128, 64], FP32, tag="psT", name="psT")
                nc.tensor.transpose(
                    psT[:, 0:64], h1[64:128, :], eye[64:128, 64:128]
                )
                # psT cols {0:16}=chunk1, {32:48}=chunk2 -> s_n cols 48k+...
                dst = s_n[:, 48 : 48 + 96].rearrange("p (k c) -> p k c", c=48)
                src = psT[:, 0:64].rearrange("p (k c) -> p k c", c=32)[:, :, 0:16]
                nc.vector.tensor_copy(dst[:, :, 0:16], src)
                nc.vector.tensor_sub(dst[:, :, 32:48], src, dst[:, :, 0:16])

            prev_h1 = None
            for t in range(t_steps):
                s_p = sP[t % 2]
                s_n = sP[(t + 1) % 2]
                h1 = houtp.tile([128, 128], FP32, tag="h1", name="h1")

                ps = {}
                acts = {}
                n3, n0, n1, n2 = ORDER  # 3, 0, 1, 2
                # G3: k3,k0 pairs; then prev step's pair-transpose (s1,s2);
                # then G3's k1,k2 pairs consume the fresh splits.
                ps[n3] = psp.tile([48, 512], FP32, tag="gates", name="psg")
                emit_mm_pairs(ps[n3], n3, s_p, ORDER[:2], start=True, stop=False)
                if prev_h1 is not None:
                    emit_t_pair12(prev_h1, s_p)
                emit_mm_pairs(ps[n3], n3, s_p, ORDER[2:], start=False, stop=True)
                acts[n3] = emit_evac(n3, ps[n3], fast=True)

                ps[n0] = psp.tile([48, 512], FP32, tag="gates", name="psg")
                emit_mm_pairs(ps[n0], n0, s_p, ORDER, start=True, stop=True)
                acts[n0] = emit_evac(n0, ps[n0], fast=True)
                emit_cchain(n3, acts[n3], h1, on_dve=True)

                ps[n1] = psp.tile([48, 512], FP32, tag="gates", name="psg")
                emit_mm_pairs(ps[n1], n1, s_p, ORDER, start=True, stop=True)
                acts[n1] = emit_evac(n1, ps[n1], fast=False)
                emit_cchain(n0, acts[n0], h1, on_dve=True)

                ps[n2] = psp.tile([48, 512], FP32, tag="gates", name="psg")
                emit_mm_pairs(ps[n2], n2, s_p, ORDER, start=True, stop=True)
                if t < t_steps - 1:
                    emit_t_single(n3, h1, s_n)
                    emit_t_single(n0, h1, s_n)
                acts[n2] = emit_evac(n2, ps[n2], fast=True)
                emit_cchain(n1, acts[n1], h1, on_dve=False)
                emit_cchain(n2, acts[n2], h1, on_dve=True)

                # ys DMA per slot: h1[32s:32s+16, :] -> ys[t, s]
                for s in range(NB):
                    nc.sync.dma_start(
                        ys_d.ap()[t, s], h1[32 * s : 32 * s + BL, :]
                    )
                prev_h1 = h1

    nc.compile()
    return nc


def _prep_host_inputs(Z, seq_len, W_ih, W_hh, b_ih, b_hh):
    """Per-core in_maps with device-native layouts."""
    WT = np.ascontiguousarray(W_hh.astype(np.float32).T)      # [H, 4H] (hid_in, gate)
    WIT = np.ascontiguousarray(W_ih.astype(np.float32).T)     # [F, 4H]
    bias = (b_ih.astype(np.float32) + b_hh.astype(np.float32))

    # column reorder: col = n*512 + r*128 + q  <->  gate index G(r)*H + 128n + q
    # with in-chunk gate order G = (i, f, o, g) so sigmoid covers cols 0:384.
    GMAP = np.array([0, 1, 3, 2])
    n_i = np.arange(2048)
    nn, rem = np.divmod(n_i, 512)
    rr, qq = np.divmod(rem, 128)
    colmap = GMAP[rr] * H + 128 * nn + qq                     # [2048]

    wr_np = np.empty((128, NB * 2048), dtype=np.float32)
    for k in range(NB):
        wr_np[:, k * 2048 : (k + 1) * 2048] = WT[k * 128 : (k + 1) * 128, colmap]
    wrh_np, wrl_np = _split12(wr_np)
    wih_np, wil_np = _split12(np.ascontiguousarray(WIT[:, colmap]))
    bias_np = np.broadcast_to(bias[colmap], (BL, 2048)).copy()
    eye_np = np.eye(128, dtype=np.float32)

    in_maps = []
    for c in range(N_CORES):
        zc = np.ascontiguousarray(Z[c * BL : (c + 1) * BL].astype(np.float32).T)
        z_hi, z_lo = _split12(zc)
        z_np = np.concatenate([z_hi, z_lo], axis=1)
        in_maps.append(
            {"wrh": wrh_np, "wrl": wrl_np, "wih": wih_np, "wil": wil_np,
             "z": z_np, "bias": bias_np, "eye": eye_np}
        )
    return in_maps


_NC_CACHE = {}


def get_nc(t_steps: int = T_STEPS):
    if t_steps not in _NC_CACHE:
        _NC_CACHE[t_steps] = build_lstm_nc(t_steps)
    return _NC_CACHE[t_steps]


def kernel(Z, seq_len, W_ih, W_hh, b_ih, b_hh, _trace=False, _tmpdir=None):
    nc = get_nc()
    in_maps = _prep_host_inputs(Z, seq_len, W_ih, W_hh, b_ih, b_hh)
    res = run_bass_kernel_spmd(
        nc, in_maps, core_ids=list(range(N_CORES)), trace=_trace, tmpdir=_tmpdir
    )
    kernel.last_result = res

    ORDER = (3, 0, 1, 2)
    out = np.zeros((B, TMAX, H), dtype=np.float32)
    for c in range(N_CORES):
        ys = res.results[c]["ys"]  # [T_STEPS, slot, BL, 128]; slot i = chunk ORDER[i]
        for s, n in enumerate(ORDER):
            out[c * BL : (c + 1) * BL, :T_STEPS, n * 128 : (n + 1) * 128] = (
                ys[:, s].transpose(1, 0, 2)
            )
    mask = np.arange(TMAX, dtype=np.int64)[None, :] < seq_len.astype(np.int64)[:, None]
    out *= mask[:, :, None].astype(np.float32)
    return out


# revision 27
# speedup vs baseline: 1.3579x; 1.0773x over previous
"""LSTM decoder (constant input per step, ragged lengths) on 8 TRN2 cores.

Math (per batch element b, for t < seq_len[b]):
    x_proj = Z @ W_ih.T + b_ih + b_hh            (constant over time)
    gates_t = x_proj + h_t @ W_hh.T
    i,f,g,o = split(gates_t); c = sig(f)*c + sig(i)*tanh(g); h = sig(o)*tanh(c)
    ys[b, t] = h_{t+1}

The recurrence is chaotic: bf16/tf32-class rounding of h or W diverges from the
fp32 reference by O(1) after ~500 steps, so products must be fp32-exact.

Device strategy (data-parallel over batch, 16 sequences per core):
  * Streaming matmul: moving = W_hh.T column blocks [128, 512] (fp32r,
    1 cycle/row at N>=256), stationary = h.T chunks, Veltkamp-split into
    ~12-bit hi/lo pieces whose products are exact in fp32.
  * M-stacking: stationary packs [hi | zeros | lo] (48 of 128 PE columns),
    so one moving pass of wh yields hi*wh (psum rows 0:16) AND lo*wh (rows
    32:48); the wl pass adds hi*wl / lo*wl.  Full exact product in 2 moving
    passes instead of 3 -> 32 matmuls/step instead of 48.
  * Row halves merge: DMA evacuates the lo half (engines stay free) while
    DVE adds x_proj to the hi half, then one DVE add combines them.
  * Gate order i|f|o|g per 128-hidden block: one sigmoid over 384 cols +
    one tanh over 128.
  * Per-chunk PE transpose + hi/lo split so the next step's first matmul
    groups (k-ordered by availability) start while late chunks' elementwise
    still runs; emission is software-pipelined to avoid engine-FIFO
    head-of-line blocking.
"""

import numpy as np

import concourse.bass as bass
import concourse.tile as tile
from concourse import bacc, mybir
from concourse.bass_utils import run_bass_kernel_spmd

B, F, H, TMAX = 128, 128, 512, 512
N_CORES = 8
BL = B // N_CORES          # local batch = 16
NB = 4                     # hidden blocks of 128 (= N chunks and K chunks)
T_STEPS = TMAX - 1         # seq_len < 512, so at most 511 steps matter
SPLIT_C = float(2.0 ** 12 + 1)

FP32 = mybir.dt.float32
FP32R = mybir.dt.float32r
AF = mybir.ActivationFunctionType


def _split12(x):
    x = x.astype(np.float32)
    v = (x * np.float32(SPLIT_C)).astype(np.float32)
    hi = (v - (v - x).astype(np.float32)).astype(np.float32)
    lo = (x - hi).astype(np.float32)
    return hi, lo


def build_lstm_nc(t_steps: int = T_STEPS):
    """Build + compile the per-core Bass program (SPMD: same NEFF, 8 cores)."""
    nc = bacc.Bacc("TRN2", target_bir_lowering=False, debug=False)

    wrh_d = nc.dram_tensor("wrh", [128, NB * 2048], FP32R, kind="ExternalInput")
    wrl_d = nc.dram_tensor("wrl", [128, NB * 2048], FP32R, kind="ExternalInput")
    wih_d = nc.dram_tensor("wih", [128, 2048], FP32R, kind="ExternalInput")
    wil_d = nc.dram_tensor("wil", [128, 2048], FP32R, kind="ExternalInput")
    z_d = nc.dram_tensor("z", [128, 48], FP32R, kind="ExternalInput")  # [hi|0|lo]
    bias_d = nc.dram_tensor("bias", [2, 2048], FP32R, kind="ExternalInput")  # hi/lo rows
    sb_d = nc.dram_tensor("sb", [2, 48], FP32R, kind="ExternalInput")  # ones selector
    eye_d = nc.dram_tensor("eye", [128, 128], FP32, kind="ExternalInput")
    # ys stored hid-block-major: [t, k, b, q] with hid = 128k + q
    ys_d = nc.dram_tensor("ys", [t_steps, NB, BL, 128], FP32, kind="ExternalOutput")

    with tile.TileContext(nc) as tc:
        with (
            tc.tile_pool(name="const", bufs=1) as constp,
            tc.tile_pool(name="state", bufs=1) as statep,
            tc.tile_pool(name="work", bufs=4) as workp,
            tc.tile_pool(name="hout", bufs=4) as houtp,
            tc.tile_pool(name="ps", bufs=4, space="PSUM") as psp,
            tc.tile_pool(name="pst", bufs=2, space="PSUM") as pstp,
        ):
            # --- constants ---
            wrh = constp.tile([128, NB * 2048], FP32R)
            nc.sync.dma_start(wrh[:], wrh_d.ap())
            wrl = constp.tile([128, NB * 2048], FP32R)
            nc.sync.dma_start(wrl[:], wrl_d.ap())
            wih = constp.tile([128, 2048], FP32R)
            nc.sync.dma_start(wih[:], wih_d.ap())
            wil = constp.tile([128, 2048], FP32R)
            nc.sync.dma_start(wil[:], wil_d.ap())
            s_z = constp.tile([128, 48], FP32R)
            nc.sync.dma_start(s_z[:], z_d.ap())
            bias2 = constp.tile([2, 2048], FP32R)
            nc.sync.dma_start(bias2[:2, :], bias_d.ap())
            s_b = constp.tile([2, 48], FP32R)
            nc.sync.dma_start(s_b[:2, :], sb_d.ap())
            eye = constp.tile([128, 128], FP32)
            nc.sync.dma_start(eye[:], eye_d.ap())

            # --- state ---
            c1 = statep.tile([BL, H], FP32)          # cell, layout 1
            nc.vector.memset(c1[:BL, :], 0.0)
            # stationary state, double buffered: sP[p] [128, 192] fp32r,
            # chunk k at cols [48k, 48k+48): hi at +0:16, ZERO at +16:32
            # (so lo's psum rows land 32-aligned), lo at +32:48.
            sP = [
                statep.tile([128, 192], FP32R, tag=f"sP{j}", name=f"sP{j}")
                for j in range(2)
            ]
            zf = statep.tile([128, 192], FP32)
            nc.vector.memset(zf[:], 0.0)
            nc.vector.tensor_copy(sP[0][:], zf[:])
            nc.vector.tensor_copy(sP[1][:], zf[:])

            # --- recurrence (software-pipelined emission) ---
            ORDER = (3, 0, 1, 2)
            SLOT = {n: i for i, n in enumerate(ORDER)}  # h1 partition slot

            def emit_zbias(ps, n):
                """x_proj + bias folded into the psum group: two moving
                passes of W_ih (z hi/lo M-stacked) + one K=2 ones-matmul
                adding bias hi+lo.  Depends on nothing from the prior step,
                so these lead each group as stall filler."""
                wi_h = wih[:, n * 512 : (n + 1) * 512]
                wi_l = wil[:, n * 512 : (n + 1) * 512]
                nc.tensor.matmul(ps[:48, :], s_z[:, 0:48], wi_h,
                                 start=True, stop=False)
                nc.tensor.matmul(ps[:48, :], s_z[:, 0:48], wi_l,
                                 start=False, stop=False)
                nc.tensor.matmul(ps[:48, :], s_b[0:2, 0:48],
                                 bias2[0:2, n * 512 : (n + 1) * 512],
                                 start=False, stop=False)

            def emit_mm_pairs(ps, n, s_p, ks, start, stop):
                for ki, k in enumerate(ks):
                    w_h = wrh[:, k * 2048 + n * 512 : k * 2048 + (n + 1) * 512]
                    w_l = wrl[:, k * 2048 + n * 512 : k * 2048 + (n + 1) * 512]
                    s_k = s_p[:, k * 48 : k * 48 + 48]
                    nc.tensor.matmul(ps[:48, :], s_k, w_h,
                                     start=(start and ki == 0), stop=False)
                    nc.tensor.matmul(ps[:48, :], s_k, w_l, start=False,
                                     stop=(stop and ki == len(ks) - 1))

            def emit_evac(n, ps, fast):
                """PSUM -> activated gates: Act evacuates the lo half, one
                DVE add merges it with the hi half (x_proj/bias already
                accumulated in PSUM by emit_zbias)."""
                lo_sb = workp.tile([BL, 512], FP32, tag="lo", name="lo_sb")
                nc.scalar.activation(lo_sb[:BL, :], ps[32:48, :], AF.Copy)
                ga = workp.tile([BL, 512], FP32, tag="ga", name="ga")
                nc.vector.tensor_add(ga[:BL, :], ps[0:BL, :], lo_sb[:BL, :])
                act = workp.tile([BL, 512], FP32, tag="act", name="act")
                nc.scalar.activation(act[:BL, 0:384], ga[:BL, 0:384], AF.Sigmoid)
                nc.scalar.activation(act[:BL, 384:512], ga[:BL, 384:512], AF.Tanh)
                return act

            def emit_cchain(n, act, h1, on_dve):
                i_s = act[:BL, 0:128]
                f_s = act[:BL, 128:256]
                o_s = act[:BL, 256:384]
                g_s = act[:BL, 384:512]
                cn = c1[:BL, n * 128 : (n + 1) * 128]
                eng = nc.vector if on_dve else nc.gpsimd
                t1 = workp.tile([BL, 128], FP32, tag="t1", name="t1")
                eng.tensor_mul(t1[:BL, :], i_s, g_s)
                eng.tensor_mul(cn, f_s, cn)
                eng.tensor_add(cn, cn, t1[:BL, :])
                tct = workp.tile([BL, 128], FP32, tag="tct", name="tct")
                nc.scalar.activation(tct[:BL, :], cn, AF.Tanh)
                hn = h1[32 * SLOT[n] : 32 * SLOT[n] + BL, :]
                nc.vector.tensor_mul(hn, o_s, tct[:BL, :])

            def emit_t_single(n, h1, s_n):
                """Transpose one chunk (slot base 0 or 32) + hi/lo split."""
                base = 32 * SLOT[n]
                psT = pstp.tile([128, 64], FP32, tag="psT", name="psT")
                nc.tensor.transpose(
                    psT[:, 0:32], h1[base : base + 32, :],
                    eye[base : base + 32, base : base + 32],
                )
                hi = s_n[:, 48 * n : 48 * n + 16]
                lo = s_n[:, 48 * n + 32 : 48 * n + 48]
                nc.vector.tensor_copy(hi, psT[:, 0:BL])
                nc.vector.tensor_sub(lo, psT[:, 0:BL], hi)

            def emit_t_pair12(h1, s_n):
                """Transpose slots 2,3 (chunks ORDER[2]=1, ORDER[3]=2) at
                base 64 together, then split both with 2-level free APs."""
                n_a, n_b = ORDER[2], ORDER[3]
                assert n_a == 1 and n_b == 2
                psT = pstp.tile([128, 64], FP32, tag="psT", name="psT")
                nc.tensor.transpose(
                    psT[:, 0:64], h1[64:128, :], eye[64:128, 64:128]
                )
                # psT cols {0:16}=chunk1, {32:48}=chunk2 -> s_n cols 48k+...
                dst = s_n[:, 48 : 48 + 96].rearrange("p (k c) -> p k c", c=48)
                src = psT[:, 0:64].rearrange("p (k c) -> p k c", c=32)[:, :, 0:16]
                nc.vector.tensor_copy(dst[:, :, 0:16], src)
                nc.vector.tensor_sub(dst[:, :, 32:48], src, dst[:, :, 0:16])

            prev_h1 = None
            for t in range(t_steps):
                s_p = sP[t % 2]
                s_n = sP[(t + 1) % 2]
                h1 = houtp.tile([128, 128], FP32, tag="h1", name="h1")

                ps = {}
                acts = {}
                n3, n0, n1, n2 = ORDER  # 3, 0, 1, 2
                # G3: k3,k0 pairs; then prev step's pair-transpose (s1,s2);
                # then G3's k1,k2 pairs consume the fresh splits.
                ps[n3] = psp.tile([48, 512], FP32, tag="gates", name="psg")
                emit_zbias(ps[n3], n3)
                emit_mm_pairs(ps[n3], n3, s_p, ORDER[:2], start=False, stop=False)
                if prev_h1 is not None:
                    emit_t_pair12(prev_h1, s_p)
                emit_mm_pairs(ps[n3], n3, s_p, ORDER[2:], start=False, stop=True)
                acts[n3] = emit_evac(n3, ps[n3], fast=True)

                ps[n0] = psp.tile([48, 512], FP32, tag="gates", name="psg")
                emit_zbias(ps[n0], n0)
                emit_mm_pairs(ps[n0], n0, s_p, ORDER, start=False, stop=True)
                acts[n0] = emit_evac(n0, ps[n0], fast=True)
                emit_cchain(n3, acts[n3], h1, on_dve=True)

                ps[n1] = psp.tile([48, 512], FP32, tag="gates", name="psg")
                emit_zbias(ps[n1], n1)
                emit_mm_pairs(ps[n1], n1, s_p, ORDER, start=False, stop=True)
                acts[n1] = emit_evac(n1, ps[n1], fast=False)
                emit_cchain(n0, acts[n0], h1, on_dve=True)

                ps[n2] = psp.tile([48, 512], FP32, tag="gates", name="psg")
                emit_zbias(ps[n2], n2)
                emit_mm_pairs(ps[n2], n2, s_p, ORDER, start=False, stop=True)
                if t < t_steps - 1:
                    emit_t_single(n3, h1, s_n)
                    emit_t_single(n0, h1, s_n)
                acts[n2] = emit_evac(n2, ps[n2], fast=True)
                emit_cchain(n1, acts[n1], h1, on_dve=False)
                emit_cchain(n2, acts[n2], h1, on_dve=True)

                # ys DMA per slot: h1[32s:32s+16, :] -> ys[t, s]
                for s in range(NB):
                    nc.sync.dma_start(
                        ys_d.ap()[t, s], h1[32 * s : 32 * s + BL, :]
                    )
                prev_h1 = h1

    nc.compile()
    return nc


def _prep_host_inputs(Z, seq_len, W_ih, W_hh, b_ih, b_hh):
    """Per-core in_maps with device-native layouts."""
    WT = np.ascontiguousarray(W_hh.astype(np.float32).T)      # [H, 4H] (hid_in, gate)
    WIT = np.ascontiguousarray(W_ih.astype(np.float32).T)     # [F, 4H]
    bias = (b_ih.astype(np.float32) + b_hh.astype(np.float32))

    # column reorder: col = n*512 + r*128 + q  <->  gate index G(r)*H + 128n + q
    # with in-chunk gate order G = (i, f, o, g) so sigmoid covers cols 0:384.
    GMAP = np.array([0, 1, 3, 2])
    n_i = np.arange(2048)
    nn, rem = np.divmod(n_i, 512)
    rr, qq = np.divmod(rem, 128)
    colmap = GMAP[rr] * H + 128 * nn + qq                     # [2048]

    wr_np = np.empty((128, NB * 2048), dtype=np.float32)
    for k in range(NB):
        wr_np[:, k * 2048 : (k + 1) * 2048] = WT[k * 128 : (k + 1) * 128, colmap]
    wrh_np, wrl_np = _split12(wr_np)
    wih_np, wil_np = _split12(np.ascontiguousarray(WIT[:, colmap]))
    b_hi, b_lo = _split12(bias[colmap])
    bias_np = np.stack([b_hi, b_lo])                          # [2, 2048]
    sb_np = np.zeros((2, 48), dtype=np.float32)
    sb_np[:, 0:16] = 1.0
    eye_np = np.eye(128, dtype=np.float32)

    in_maps = []
    for c in range(N_CORES):
        zc = np.ascontiguousarray(Z[c * BL : (c + 1) * BL].astype(np.float32).T)
        z_hi, z_lo = _split12(zc)
        z_np = np.zeros((128, 48), dtype=np.float32)
        z_np[:, 0:16] = z_hi
        z_np[:, 32:48] = z_lo
        in_maps.append(
            {"wrh": wrh_np, "wrl": wrl_np, "wih": wih_np, "wil": wil_np,
             "z": z_np, "bias": bias_np, "eye": eye_np, "sb": sb_np}
        )
    return in_maps


_NC_CACHE = {}


def get_nc(t_steps: int = T_STEPS):
    if t_steps not in _NC_CACHE:
        _NC_CACHE[t_steps] = build_lstm_nc(t_steps)
    return _NC_CACHE[t_steps]


def kernel(Z, seq_len, W_ih, W_hh, b_ih, b_hh, _trace=False, _tmpdir=None):
    nc = get_nc()
    in_maps = _prep_host_inputs(Z, seq_len, W_ih, W_hh, b_ih, b_hh)
    res = run_bass_kernel_spmd(
        nc, in_maps, core_ids=list(range(N_CORES)), trace=_trace, tmpdir=_tmpdir
    )
    kernel.last_result = res

    ORDER = (3, 0, 1, 2)
    out = np.zeros((B, TMAX, H), dtype=np.float32)
    for c in range(N_CORES):
        ys = res.results[c]["ys"]  # [T_STEPS, slot, BL, 128]; slot i = chunk ORDER[i]
        for s, n in enumerate(ORDER):
            out[c * BL : (c + 1) * BL, :T_STEPS, n * 128 : (n + 1) * 128] = (
                ys[:, s].transpose(1, 0, 2)
            )
    mask = np.arange(TMAX, dtype=np.int64)[None, :] < seq_len.astype(np.int64)[:, None]
    out *= mask[:, :, None].astype(np.float32)
    return out


# revision 29
# speedup vs baseline: 1.3653x; 1.0055x over previous
"""LSTM decoder (constant input per step, ragged lengths) on 8 TRN2 cores.

Math (per batch element b, for t < seq_len[b]):
    x_proj = Z @ W_ih.T + b_ih + b_hh            (constant over time)
    gates_t = x_proj + h_t @ W_hh.T
    i,f,g,o = split(gates_t); c = sig(f)*c + sig(i)*tanh(g); h = sig(o)*tanh(c)
    ys[b, t] = h_{t+1}

The recurrence is chaotic: bf16/tf32-class rounding of h or W diverges from the
fp32 reference by O(1) after ~500 steps, so products must be fp32-exact.

Device strategy (data-parallel over batch, 16 sequences per core):
  * Streaming matmul: moving = W_hh.T column blocks [128, 512] (fp32r,
    1 cycle/row at N>=256), stationary = h.T chunks, Veltkamp-split into
    ~12-bit hi/lo pieces whose products are exact in fp32.
  * M-stacking: stationary packs [hi | zeros | lo] (48 of 128 PE columns),
    so one moving pass of wh yields hi*wh (psum rows 0:16) AND lo*wh (rows
    32:48); the wl pass adds hi*wl / lo*wl.  Full exact product in 2 moving
    passes instead of 3 -> 32 matmuls/step instead of 48.
  * Row halves merge: DMA evacuates the lo half (engines stay free) while
    DVE adds x_proj to the hi half, then one DVE add combines them.
  * Gate order i|f|o|g per 128-hidden block: one sigmoid over 384 cols +
    one tanh over 128.
  * Per-chunk PE transpose + hi/lo split so the next step's first matmul
    groups (k-ordered by availability) start while late chunks' elementwise
    still runs; emission is software-pipelined to avoid engine-FIFO
    head-of-line blocking.
"""

import numpy as np

import concourse.bass as bass
import concourse.tile as tile
from concourse import bacc, mybir
from concourse.bass_utils import run_bass_kernel_spmd

B, F, H, TMAX = 128, 128, 512, 512
N_CORES = 8
BL = B // N_CORES          # local batch = 16
NB = 4                     # hidden blocks of 128 (= N chunks and K chunks)
T_STEPS = TMAX - 1         # seq_len < 512, so at most 511 steps matter
SPLIT_C = float(2.0 ** 12 + 1)

FP32 = mybir.dt.float32
FP32R = mybir.dt.float32r
AF = mybir.ActivationFunctionType


def _split12(x):
    x = x.astype(np.float32)
    v = (x * np.float32(SPLIT_C)).astype(np.float32)
    hi = (v - (v - x).astype(np.float32)).astype(np.float32)
    lo = (x - hi).astype(np.float32)
    return hi, lo


def build_lstm_nc(t_steps: int = T_STEPS):
    """Build + compile the per-core Bass program (SPMD: same NEFF, 8 cores)."""
    nc = bacc.Bacc("TRN2", target_bir_lowering=False, debug=False)

    wrh_d = nc.dram_tensor("wrh", [128, NB * 2048], FP32R, kind="ExternalInput")
    wrl_d = nc.dram_tensor("wrl", [128, NB * 2048], FP32R, kind="ExternalInput")
    wih_d = nc.dram_tensor("wih", [128, 2048], FP32R, kind="ExternalInput")
    wil_d = nc.dram_tensor("wil", [128, 2048], FP32R, kind="ExternalInput")
    z_d = nc.dram_tensor("z", [128, 48], FP32R, kind="ExternalInput")  # [hi|0|lo]
    bias_d = nc.dram_tensor("bias", [2, 2048], FP32R, kind="ExternalInput")  # hi/lo rows
    sb_d = nc.dram_tensor("sb", [2, 48], FP32R, kind="ExternalInput")  # ones selector
    eye_d = nc.dram_tensor("eye", [128, 128], FP32, kind="ExternalInput")
    # ys stored hid-block-major: [t, k, b, q] with hid = 128k + q
    ys_d = nc.dram_tensor("ys", [t_steps, NB, BL, 128], FP32, kind="ExternalOutput")

    with tile.TileContext(nc) as tc:
        with (
            tc.tile_pool(name="const", bufs=1) as constp,
            tc.tile_pool(name="state", bufs=1) as statep,
            tc.tile_pool(name="work", bufs=4) as workp,
            tc.tile_pool(name="hout", bufs=4) as houtp,
            tc.tile_pool(name="ps", bufs=4, space="PSUM") as psp,
            tc.tile_pool(name="pst", bufs=2, space="PSUM") as pstp,
        ):
            # --- constants ---
            wrh = constp.tile([128, NB * 2048], FP32R)
            nc.sync.dma_start(wrh[:], wrh_d.ap())
            wrl = constp.tile([128, NB * 2048], FP32R)
            nc.sync.dma_start(wrl[:], wrl_d.ap())
            wih = constp.tile([128, 2048], FP32R)
            nc.sync.dma_start(wih[:], wih_d.ap())
            wil = constp.tile([128, 2048], FP32R)
            nc.sync.dma_start(wil[:], wil_d.ap())
            s_z = constp.tile([128, 48], FP32R)
            nc.sync.dma_start(s_z[:], z_d.ap())
            bias2 = constp.tile([2, 2048], FP32R)
            nc.sync.dma_start(bias2[:2, :], bias_d.ap())
            s_b = constp.tile([2, 48], FP32R)
            nc.sync.dma_start(s_b[:2, :], sb_d.ap())
            eye = constp.tile([128, 128], FP32)
            nc.sync.dma_start(eye[:], eye_d.ap())

            # --- state ---
            c1 = statep.tile([BL, H], FP32)          # cell, layout 1
            nc.vector.memset(c1[:BL, :], 0.0)
            # stationary state, double buffered: sP[p] [128, 192] fp32r,
            # chunk k at cols [48k, 48k+48): hi at +0:16, ZERO at +16:32
            # (so lo's psum rows land 32-aligned), lo at +32:48.
            sP = [
                statep.tile([128, 192], FP32R, tag=f"sP{j}", name=f"sP{j}")
                for j in range(2)
            ]
            zf = statep.tile([128, 192], FP32)
            nc.vector.memset(zf[:], 0.0)
            nc.vector.tensor_copy(sP[0][:], zf[:])
            nc.vector.tensor_copy(sP[1][:], zf[:])

            # --- recurrence (software-pipelined emission) ---
            ORDER = (3, 0, 1, 2)
            SLOT = {n: i for i, n in enumerate(ORDER)}  # h1 partition slot

            def emit_zbias(ps, n):
                """x_proj + bias folded into the psum group: two moving
                passes of W_ih (z hi/lo M-stacked) + one K=2 ones-matmul
                adding bias hi+lo.  Depends on nothing from the prior step,
                so these lead each group as stall filler."""
                wi_h = wih[:, n * 512 : (n + 1) * 512]
                wi_l = wil[:, n * 512 : (n + 1) * 512]
                nc.tensor.matmul(ps[:48, :], s_z[:, 0:48], wi_h,
                                 start=True, stop=False)
                nc.tensor.matmul(ps[:48, :], s_z[:, 0:48], wi_l,
                                 start=False, stop=False)
                nc.tensor.matmul(ps[:48, :], s_b[0:2, 0:48],
                                 bias2[0:2, n * 512 : (n + 1) * 512],
                                 start=False, stop=False)

            def emit_mm_pairs(ps, n, s_p, ks, start, stop):
                for ki, k in enumerate(ks):
                    w_h = wrh[:, k * 2048 + n * 512 : k * 2048 + (n + 1) * 512]
                    w_l = wrl[:, k * 2048 + n * 512 : k * 2048 + (n + 1) * 512]
                    s_k = s_p[:, k * 48 : k * 48 + 48]
                    nc.tensor.matmul(ps[:48, :], s_k, w_h,
                                     start=(start and ki == 0), stop=False)
                    nc.tensor.matmul(ps[:48, :], s_k, w_l, start=False,
                                     stop=(stop and ki == len(ks) - 1))

            def emit_evac(n, ps, fast):
                """PSUM -> activated gates: Act evacuates the lo half, one
                DVE add merges it with the hi half (x_proj/bias already
                accumulated in PSUM by emit_zbias)."""
                lo_sb = workp.tile([BL, 512], FP32, tag="lo", name="lo_sb")
                nc.scalar.activation(lo_sb[:BL, :], ps[32:48, :], AF.Copy)
                ga = workp.tile([BL, 512], FP32, tag="ga", name="ga")
                nc.vector.tensor_add(ga[:BL, :], ps[0:BL, :], lo_sb[:BL, :])
                act = workp.tile([BL, 512], FP32, tag="act", name="act")
                nc.scalar.activation(act[:BL, 0:384], ga[:BL, 0:384], AF.Sigmoid)
                nc.scalar.activation(act[:BL, 384:512], ga[:BL, 384:512], AF.Tanh)
                return act

            def emit_cchain(n, act, h1, on_dve):
                i_s = act[:BL, 0:128]
                f_s = act[:BL, 128:256]
                o_s = act[:BL, 256:384]
                g_s = act[:BL, 384:512]
                cn = c1[:BL, n * 128 : (n + 1) * 128]
                eng = nc.vector if on_dve else nc.gpsimd
                t1 = workp.tile([BL, 128], FP32, tag="t1", name="t1")
                eng.tensor_mul(t1[:BL, :], i_s, g_s)
                eng.tensor_mul(cn, f_s, cn)
                eng.tensor_add(cn, cn, t1[:BL, :])
                tct = workp.tile([BL, 128], FP32, tag="tct", name="tct")
                nc.scalar.activation(tct[:BL, :], cn, AF.Tanh)
                hn = h1[32 * SLOT[n] : 32 * SLOT[n] + BL, :]
                nc.vector.tensor_mul(hn, o_s, tct[:BL, :])

            def emit_t_single(n, h1, s_n):
                """Transpose one chunk (slot base 0 or 32) + hi/lo split."""
                base = 32 * SLOT[n]
                psT = pstp.tile([128, 64], FP32, tag="psT", name="psT")
                nc.tensor.transpose(
                    psT[:, 0:32], h1[base : base + 32, :],
                    eye[base : base + 32, base : base + 32],
                )
                hi = s_n[:, 48 * n : 48 * n + 16]
                lo = s_n[:, 48 * n + 32 : 48 * n + 48]
                nc.vector.tensor_copy(hi, psT[:, 0:BL])
                nc.vector.tensor_sub(lo, psT[:, 0:BL], hi)

            def emit_t_pair12(h1, s_n):
                """Transpose slots 2,3 (chunks ORDER[2]=1, ORDER[3]=2) at
                base 64 together, then split both with 2-level free APs."""
                n_a, n_b = ORDER[2], ORDER[3]
                assert n_a == 1 and n_b == 2
                psT = pstp.tile([128, 64], FP32, tag="psT", name="psT")
                nc.tensor.transpose(
                    psT[:, 0:64], h1[64:128, :], eye[64:128, 64:128]
                )
                # psT cols {0:16}=chunk1, {32:48}=chunk2 -> s_n cols 48k+...
                dst = s_n[:, 48 : 48 + 96].rearrange("p (k c) -> p k c", c=48)
                src = psT[:, 0:64].rearrange("p (k c) -> p k c", c=32)[:, :, 0:16]
                nc.vector.tensor_copy(dst[:, :, 0:16], src)
                nc.vector.tensor_sub(dst[:, :, 32:48], src, dst[:, :, 0:16])

            prev_h1 = None
            for t in range(t_steps):
                s_p = sP[t % 2]
                s_n = sP[(t + 1) % 2]
                h1 = houtp.tile([128, 128], FP32, tag="h1", name="h1")

                ps = {}
                acts = {}
                n3, n0, n1, n2 = ORDER  # 3, 0, 1, 2
                # G3: k3,k0 pairs; then prev step's pair-transpose (s1,s2);
                # then G3's k1,k2 pairs consume the fresh splits.
                ps[n3] = psp.tile([48, 512], FP32, tag="gates", name="psg")
                emit_zbias(ps[n3], n3)
                emit_mm_pairs(ps[n3], n3, s_p, ORDER[:2], start=False, stop=False)
                if prev_h1 is not None:
                    emit_t_pair12(prev_h1, s_p)
                emit_mm_pairs(ps[n3], n3, s_p, ORDER[2:], start=False, stop=True)
                acts[n3] = emit_evac(n3, ps[n3], fast=True)

                ps[n0] = psp.tile([48, 512], FP32, tag="gates", name="psg")
                emit_zbias(ps[n0], n0)
                emit_mm_pairs(ps[n0], n0, s_p, ORDER, start=False, stop=True)
                acts[n0] = emit_evac(n0, ps[n0], fast=True)
                emit_cchain(n3, acts[n3], h1, on_dve=True)

                ps[n1] = psp.tile([48, 512], FP32, tag="gates", name="psg")
                emit_zbias(ps[n1], n1)
                emit_mm_pairs(ps[n1], n1, s_p, ORDER, start=False, stop=True)
                acts[n1] = emit_evac(n1, ps[n1], fast=False)
                emit_cchain(n0, acts[n0], h1, on_dve=True)

                ps[n2] = psp.tile([48, 512], FP32, tag="gates", name="psg")
                emit_zbias(ps[n2], n2)
                emit_mm_pairs(ps[n2], n2, s_p, ORDER, start=False, stop=True)
                if t < t_steps - 1:
                    emit_t_single(n3, h1, s_n)
                    emit_t_single(n0, h1, s_n)
                acts[n2] = emit_evac(n2, ps[n2], fast=True)
                emit_cchain(n1, acts[n1], h1, on_dve=False)
                emit_cchain(n2, acts[n2], h1, on_dve=True)

                # ys DMA per slot: h1[32s:32s+16, :] -> ys[t, s]
                for s in range(NB):
                    nc.sync.dma_start(
                        ys_d.ap()[t, s], h1[32 * s : 32 * s + BL, :]
                    )
                prev_h1 = h1

    nc.compile()
    return nc


def _prep_host_inputs(Z, seq_len, W_ih, W_hh, b_ih, b_hh):
    """Per-core in_maps with device-native layouts."""
    WT = np.ascontiguousarray(W_hh.astype(np.float32).T)      # [H, 4H] (hid_in, gate)
    WIT = np.ascontiguousarray(W_ih.astype(np.float32).T)     # [F, 4H]
    bias = (b_ih.astype(np.float32) + b_hh.astype(np.float32))

    # column reorder: col = n*512 + r*128 + q  <->  gate index G(r)*H + 128n + q
    # with in-chunk gate order G = (i, f, o, g) so sigmoid covers cols 0:384.
    GMAP = np.array([0, 1, 3, 2])
    n_i = np.arange(2048)
    nn, rem = np.divmod(n_i, 512)
    rr, qq = np.divmod(rem, 128)
    colmap = GMAP[rr] * H + 128 * nn + qq                     # [2048]

    wr_np = np.empty((128, NB * 2048), dtype=np.float32)
    for k in range(NB):
        wr_np[:, k * 2048 : (k + 1) * 2048] = WT[k * 128 : (k + 1) * 128, colmap]
    wrh_np, wrl_np = _split12(wr_np)
    wih_np, wil_np = _split12(np.ascontiguousarray(WIT[:, colmap]))
    b_hi, b_lo = _split12(bias[colmap])
    bias_np = np.stack([b_hi, b_lo])                          # [2, 2048]
    sb_np = np.zeros((2, 48), dtype=np.float32)
    sb_np[:, 0:16] = 1.0
    eye_np = np.eye(128, dtype=np.float32)

    in_maps = []
    for c in range(N_CORES):
        zc = np.ascontiguousarray(Z[c * BL : (c + 1) * BL].astype(np.float32).T)
        z_hi, z_lo = _split12(zc)
        z_np = np.zeros((128, 48), dtype=np.float32)
        z_np[:, 0:16] = z_hi
        z_np[:, 32:48] = z_lo
        in_maps.append(
            {"wrh": wrh_np, "wrl": wrl_np, "wih": wih_np, "wil": wil_np,
             "z": z_np, "bias": bias_np, "eye": eye_np, "sb": sb_np}
        )
    return in_maps


_NC_CACHE = {}


def get_nc(t_steps: int = T_STEPS):
    if t_steps not in _NC_CACHE:
        _NC_CACHE[t_steps] = build_lstm_nc(t_steps)
    return _NC_CACHE[t_steps]


def kernel(Z, seq_len, W_ih, W_hh, b_ih, b_hh, _trace=False, _tmpdir=None):
    nc = get_nc()
    in_maps = _prep_host_inputs(Z, seq_len, W_ih, W_hh, b_ih, b_hh)
    res = run_bass_kernel_spmd(
        nc, in_maps, core_ids=list(range(N_CORES)), trace=_trace, tmpdir=_tmpdir
    )
    kernel.last_result = res

    ORDER = (3, 0, 1, 2)
    out = np.zeros((B, TMAX, H), dtype=np.float32)
    for c in range(N_CORES):
        ys = res.results[c]["ys"]  # [T_STEPS, slot, BL, 128]; slot i = chunk ORDER[i]
        for s, n in enumerate(ORDER):
            out[c * BL : (c + 1) * BL, :T_STEPS, n * 128 : (n + 1) * 128] = (
                ys[:, s].transpose(1, 0, 2)
            )
    mask = np.arange(TMAX, dtype=np.int64)[None, :] < seq_len.astype(np.int64)[:, None]
    out *= mask[:, :, None].astype(np.float32)
    return out
